# revision 4
# baseline (speedup 1.0000x reference)
"""Trainium2 Bass kernel for nn_EnvironmentalAugmentations.

Computes, for waveform/white_noise of shape [256, 220500] fp32:
    pink  = first-order IIR of white_noise along time:
            f[0] = w[0];  f[t] = 0.99*f[t-1] + 0.01*w[t]
    mixed = waveform + 0.05 * pink
    out   = mixed / max(max|mixed|, 1.0)     (global max over all elements)

Strategy (8 NeuronCores, pure data-parallel over the 256 channels, 32/core):
  * Each channel's T=220500 samples are laid out as an SBUF tile
    [126 partitions x 1750], partition p holding samples [p*1750,(p+1)*1750).
  * The IIR runs as ONE DVE `tensor_tensor_scan` per channel (state =
    a*state + w[t]), independently per partition (zero initial state).
  * Cross-partition carry: the true state entering partition p is the last
    scan value of partition p-1 (the a^1750 ~ 2e-8 attenuated remainder is
    below fp32 noise).  A tiny PE matmul with a superdiagonal shift matrix
    moves column L-1 down one partition; a fused scalar_tensor_tensor adds
    carry*a^(k+1)*0.05*b + waveform, a second one adds 0.05*b*scan.
  * The t=0 initial condition (f[0]=w[0], coefficient 1 not b) is a rank-1
    fixup on partition 0 only.
  * Global max: per-channel abs-max reduces -> column tile -> DVE reduce ->
    GPSIMD partition_all_reduce -> 8-core AllReduce(max) on a 4-byte DRAM
    scratch -> scale = 1/max(m, 1.0) broadcast -> second pass rescales.
"""

import numpy as np

# fp32-rounded constants, mirrored from the reference
_A = float(np.float32(0.99))
_B = float(np.float32(0.01))
_NOISE = float(np.float32(0.05))

C_FULL, T_FULL = 256, 220500
N_CORES = 8
C_PER = C_FULL // N_CORES  # 32
P_USED = 126
L = T_FULL // P_USED       # 1750  (126*1750 == 220500 exactly)


def _host_consts(p_used, l):
    """decay05[p,k] = 0.05*b*a^(k+1) replicated over partitions; shift matrix."""
    a64, b64, n64 = _A, _B, _NOISE
    k = np.arange(1, l + 1, dtype=np.float64)
    decay_row = (n64 * b64) * np.power(a64, k)
    decay = np.ascontiguousarray(
        np.broadcast_to(decay_row.astype(np.float32), (p_used, l))
    )
    shift = np.zeros((p_used, p_used), dtype=np.float32)
    shift[np.arange(p_used - 1), np.arange(p_used - 1) + 1] = 1.0
    return decay, shift


def build_nc(c_per=C_PER, p_used=P_USED, l=L, n_cores=N_CORES):
    """Build the Bass module (per-core SPMD program)."""
    import concourse.mybir as mybir
    from concourse import bacc, bass_isa
    from concourse.tile import TileContext

    f32 = mybir.dt.float32
    Alu = mybir.AluOpType
    AxX = mybir.AxisListType.X
    t_loc = p_used * l

    # host-side scalar constants (float64 of the f32-rounded refs)
    K0 = (1.0 - _B) / (_B * _A)     # p0 fixup: w00 coefficient / decay05
    SC2 = _NOISE * _B               # 0.05*b, the scan-output coefficient

    nc = bacc.Bacc(
        "TRN2", target_bir_lowering=False, debug=False, num_devices=n_cores
    )
    wave_h = nc.dram_tensor("waveform", [c_per, t_loc], f32, kind="ExternalInput")
    noise_h = nc.dram_tensor("white_noise", [c_per, t_loc], f32, kind="ExternalInput")
    decay_h = nc.dram_tensor("decay05", [p_used, l], f32, kind="ExternalInput")
    shift_h = nc.dram_tensor("shiftmat", [p_used, p_used], f32, kind="ExternalInput")
    out_h = nc.dram_tensor("out", [c_per, t_loc], f32, kind="ExternalOutput")

    wave_r = wave_h.rearrange("c (p l) -> c p l", p=p_used)
    noise_r = noise_h.rearrange("c (p l) -> c p l", p=p_used)
    out_r = out_h.rearrange("c (p l) -> c p l", p=p_used)

    with TileContext(nc) as tc:
        with (
            tc.tile_pool(name="const", bufs=1) as constp,
            tc.tile_pool(name="dram", bufs=1, space="DRAM") as dramp,
        ):
            # constants
            a_tile = constp.tile([p_used, l], f32, tag="a_tile")
            nc.gpsimd.memset(a_tile[:], _A)
            decay_t = constp.tile([p_used, l], f32, tag="decay")
            nc.sync.dma_start(out=decay_t[:], in_=decay_h[:, :])
            shift_t = constp.tile([p_used, p_used], f32, tag="shift")
            nc.sync.dma_start(out=shift_t[:], in_=shift_h[:, :])
            maxcols = constp.tile([p_used, c_per], f32, tag="maxcols")

            mixed_drams = []
            with (
                tc.tile_pool(name="io1", bufs=3) as iop,
                tc.tile_pool(name="work", bufs=2) as workp,
                tc.tile_pool(name="psum", bufs=2, space="PSUM") as psump,
            ):
                for c in range(c_per):
                    w_t = iop.tile([p_used, l], f32, tag="w")
                    nc.sync.dma_start(out=w_t[:], in_=noise_r[c])
                    wav_t = iop.tile([p_used, l], f32, tag="wav")
                    nc.sync.dma_start(out=wav_t[:], in_=wave_r[c])

                    # g[p,k] = sum_{j<=k} a^(k-j) * w[p,j]   (per-partition IIR)
                    g_t = workp.tile([p_used, l], f32, tag="g")
                    nc.vector.tensor_tensor_scan(
                        g_t[:], a_tile[:], w_t[:], 0.0, Alu.mult, Alu.add
                    )

                    # carry[p] = g[p-1, L-1] (PE shift), carry[0] = 0
                    carry_ps = psump.tile([p_used, 1], f32, tag="carry")
                    nc.tensor.matmul(
                        carry_ps[:], shift_t[:], g_t[:, l - 1 : l],
                        start=True, stop=True,
                    )

                    # s1 = decay05*carry + waveform
                    s1_t = workp.tile([p_used, l], f32, tag="s1")
                    nc.vector.scalar_tensor_tensor(
                        s1_t[:], decay_t[:], carry_ps[:, 0:1], wav_t[:],
                        Alu.mult, Alu.add,
                    )
                    # partition-0 fixup for the f[0]=w[0] initial condition:
                    # s1[0,k] += decay05[k] * K0 * w[0,0]
                    t0_t = workp.tile([1, 1], f32, tag="t0")
                    nc.vector.tensor_scalar_mul(t0_t[:], w_t[0:1, 0:1], float(K0))
                    nc.vector.scalar_tensor_tensor(
                        s1_t[0:1, :], decay_t[0:1, :], t0_t[0:1, 0:1], s1_t[0:1, :],
                        Alu.mult, Alu.add,
                    )

                    # mixed = 0.05*b*g + s1
                    mixed_t = workp.tile([p_used, l], f32, tag="mixed")
                    nc.vector.scalar_tensor_tensor(
                        mixed_t[:], g_t[:], float(SC2), s1_t[:], Alu.mult, Alu.add
                    )

                    # per-channel abs-max -> column c
                    nc.vector.tensor_reduce(
                        maxcols[:, c : c + 1], mixed_t[:], AxX, Alu.max,
                        apply_absolute_value=True,
                    )

                    md = dramp.tile([p_used, l], f32, tag=f"mix{c}")
                    nc.sync.dma_start(out=md[:], in_=mixed_t[:])
                    mixed_drams.append(md)

                # ---- global max + scale ----
                allmax = constp.tile([p_used, 1], f32, tag="allmax")
                nc.vector.tensor_reduce(
                    allmax[:], maxcols[:, 0:c_per], AxX, Alu.max
                )
                gmax = constp.tile([p_used, 1], f32, tag="gmax")
                nc.gpsimd.partition_all_reduce(
                    gmax[:], allmax[:], channels=p_used,
                    reduce_op=bass_isa.ReduceOp.max,
                )
                sc_b = constp.tile([p_used, 1], f32, tag="scb")
                if n_cores > 1:
                    cc_in = dramp.tile([1, 1], f32, tag="ccin")
                    cc_out = dramp.tile([1, 1], f32, tag="ccout")
                    nc.sync.dma_start(out=cc_in[:], in_=gmax[0:1, 0:1])
                    nc.gpsimd.collective_compute(
                        "AllReduce",
                        Alu.max,
                        replica_groups=[list(range(n_cores))],
                        ins=[cc_in[:]],
                        outs=[cc_out[:]],
                    )
                    sc_small = constp.tile([1, 1], f32, tag="scsmall")
                    nc.sync.dma_start(out=sc_small[:], in_=cc_out[:])
                    nc.gpsimd.partition_broadcast(
                        sc_b[:], sc_small[0:1, 0:1], channels=p_used
                    )
                else:
                    nc.vector.tensor_copy(sc_b[:], gmax[:])
                # scale = 1 / max(gmax, 1.0)
                nc.vector.tensor_scalar_max(sc_b[:], sc_b[:], 1.0)
                inv_t = constp.tile([p_used, 1], f32, tag="inv")
                nc.vector.reciprocal(inv_t[:], sc_b[:])

            # ---- phase 2: rescale ----
            with tc.tile_pool(name="io2", bufs=3) as iop2:
                for c in range(c_per):
                    m_t = iop2.tile([p_used, l], f32, tag="m2")
                    nc.sync.dma_start(out=m_t[:], in_=mixed_drams[c][:])
                    o_t = iop2.tile([p_used, l], f32, tag="o2")
                    nc.vector.tensor_scalar_mul(o_t[:], m_t[:], inv_t[:, 0:1])
                    nc.sync.dma_start(out=out_r[c], in_=o_t[:])

    nc.compile()
    return nc


_CACHE = {}
LAST_RESULTS = None


def run(waveform, white_noise, c_per=C_PER, p_used=P_USED, l=L, n_cores=N_CORES,
        **spmd_kwargs):
    """Shard inputs over n_cores, run the SPMD bass kernel, gather output."""
    global LAST_RESULTS
    from concourse.bass_utils import run_bass_kernel_spmd

    key = (c_per, p_used, l, n_cores)
    if key not in _CACHE:
        _CACHE[key] = build_nc(c_per, p_used, l, n_cores)
    nc = _CACHE[key]

    decay, shift = _host_consts(p_used, l)
    waveform = np.ascontiguousarray(waveform, dtype=np.float32)
    white_noise = np.ascontiguousarray(white_noise, dtype=np.float32)

    in_maps = []
    for i in range(n_cores):
        sl = slice(i * c_per, (i + 1) * c_per)
        in_maps.append({
            "waveform": np.ascontiguousarray(waveform[sl]),
            "white_noise": np.ascontiguousarray(white_noise[sl]),
            "decay05": decay,
            "shiftmat": shift,
        })

    res = run_bass_kernel_spmd(nc, in_maps, core_ids=list(range(n_cores)),
                               **spmd_kwargs)
    LAST_RESULTS = res
    return np.concatenate([r["out"] for r in res.results], axis=0)


def kernel(waveform, white_noise):
    return run(waveform, white_noise)


# revision 6
# speedup vs baseline: 1.0830x; 1.0830x over previous
"""Trainium2 Bass kernel for nn_EnvironmentalAugmentations.

Computes, for waveform/white_noise of shape [256, 220500] fp32:
    pink  = first-order IIR of white_noise along time:
            f[0] = w[0];  f[t] = 0.99*f[t-1] + 0.01*w[t]
    mixed = waveform + 0.05 * pink
    out   = mixed / max(max|mixed|, 1.0)     (global max over all elements)

Strategy (8 NeuronCores, pure data-parallel over the 256 channels, 32/core):
  * Each channel's T=220500 samples live in an SBUF tile [126 x 1750],
    partition p holding samples [p*1750, (p+1)*1750).
  * The IIR runs as ONE DVE `tensor_tensor_scan` per channel
    (state = a*state + w[t]), independently per partition (zero init).
  * Cross-partition carry: the true state entering partition p is the last
    scan value of partition p-1 (the a^1750 ~ 2e-8 attenuated remainder is
    below fp32 noise).  A PE matmul with a superdiagonal shift matrix moves
    column L-1 down one partition; a second accumulated matmul injects the
    t=0 initial-condition term K0*w[0,0] into partition 0 of the same PSUM
    column.  A fused scalar_tensor_tensor then computes
    s1 = decay05*carry + waveform, and a second one mixed = 0.05b*g + s1.
  * Global max: per-group abs-max reduce -> column tile -> DVE reduce ->
    GPSIMD partition_all_reduce -> 8-core AllReduce(max) on a 4-byte DRAM
    scratch -> scale = 1/max(m,1) -> ScalarE rescale pass (phase 2).
  * DMA is split over the three descriptor engines (sync/scalar HWDGE +
    gpsimd SWDGE) in 2-channel 1.76MB transfers to overlap fixed costs.
"""

import numpy as np

# fp32-rounded constants, mirrored from the reference
_A = float(np.float32(0.99))
_B = float(np.float32(0.01))
_NOISE = float(np.float32(0.05))

C_FULL, T_FULL = 256, 220500
N_CORES = 8
C_PER = C_FULL // N_CORES  # 32
P_USED = 126
L = T_FULL // P_USED       # 1750  (126*1750 == 220500 exactly)
GRP = 2                    # channels per DMA/elementwise group


def _host_consts(p_used, l):
    """decay05[p,k] = 0.05*b*a^(k+1) replicated over partitions; shift matrix."""
    a64, b64, n64 = _A, _B, _NOISE
    k = np.arange(1, l + 1, dtype=np.float64)
    decay_row = (n64 * b64) * np.power(a64, k)
    decay = np.ascontiguousarray(
        np.broadcast_to(decay_row.astype(np.float32), (p_used, l))
    )
    shift = np.zeros((p_used, p_used), dtype=np.float32)
    shift[np.arange(p_used - 1), np.arange(p_used - 1) + 1] = 1.0
    return decay, shift


def build_nc(c_per=C_PER, p_used=P_USED, l=L, n_cores=N_CORES, grp=GRP):
    """Build the Bacc module (per-core SPMD program)."""
    import concourse.mybir as mybir
    from concourse import bacc, bass_isa
    from concourse.tile import TileContext

    f32 = mybir.dt.float32
    Alu = mybir.AluOpType
    AxX = mybir.AxisListType.X
    t_loc = p_used * l
    assert c_per % grp == 0
    n_grp = c_per // grp

    # host-side scalar constants (float64 of the f32-rounded refs)
    K0 = (1.0 - _B) / (_B * _A)     # p0 fixup: carry engineered to K0*w[0,0]
    SC2 = _NOISE * _B               # 0.05*b, the scan-output coefficient

    nc = bacc.Bacc(
        "TRN2", target_bir_lowering=False, debug=False, num_devices=n_cores
    )
    wave_h = nc.dram_tensor("waveform", [c_per, t_loc], f32, kind="ExternalInput")
    noise_h = nc.dram_tensor("white_noise", [c_per, t_loc], f32, kind="ExternalInput")
    decay_h = nc.dram_tensor("decay05", [p_used, l], f32, kind="ExternalInput")
    shift_h = nc.dram_tensor("shiftmat", [p_used, p_used], f32, kind="ExternalInput")
    out_h = nc.dram_tensor("out", [c_per, t_loc], f32, kind="ExternalOutput")

    # [p, c, l] views for grouped per-channel-block DMAs
    wave_pr = wave_h.rearrange("c (p l) -> p c l", p=p_used)
    noise_pr = noise_h.rearrange("c (p l) -> p c l", p=p_used)
    out_pr = out_h.rearrange("c (p l) -> p c l", p=p_used)

    with TileContext(nc) as tc:
        with (
            tc.tile_pool(name="const", bufs=1) as constp,
            tc.tile_pool(name="dram", bufs=1, space="DRAM") as dramp,
        ):
            # constants
            a_tile = constp.tile([p_used, l], f32, tag="a_tile")
            nc.gpsimd.memset(a_tile[:], _A)
            decay_t = constp.tile([p_used, l], f32, tag="decay")
            nc.sync.dma_start(out=decay_t[:], in_=decay_h[:, :])
            shift_t = constp.tile([p_used, p_used], f32, tag="shift")
            nc.sync.dma_start(out=shift_t[:], in_=shift_h[:, :])
            # e0k0[0, m] = K0 * (m == 0): second (accumulated) matmul injects
            # carry[0] = K0 * w[0,0]
            e0k0 = constp.tile([1, p_used], f32, tag="e0k0")
            nc.gpsimd.memset(e0k0[:], 0.0)
            nc.gpsimd.memset(e0k0[0:1, 0:1], float(K0))
            maxcols = constp.tile([p_used, n_grp], f32, tag="maxcols")

            mixed_drams = []
            with (
                tc.tile_pool(name="io1", bufs=3) as iop,
                tc.tile_pool(name="work", bufs=2) as workp,
                tc.tile_pool(name="psum", bufs=4, space="PSUM") as psump,
            ):
                for g in range(n_grp):
                    cs = slice(g * grp, (g + 1) * grp)
                    w_t = iop.tile([p_used, grp * l], f32, tag="w")
                    nc.sync.dma_start(
                        out=w_t.rearrange("p (c l) -> p c l", c=grp),
                        in_=noise_pr[:, cs, :],
                    )
                    wav_t = iop.tile([p_used, grp * l], f32, tag="wav")
                    nc.scalar.dma_start(
                        out=wav_t.rearrange("p (c l) -> p c l", c=grp),
                        in_=wave_pr[:, cs, :],
                    )

                    g_t = workp.tile([p_used, grp * l], f32, tag="g")
                    carry_ps = []
                    for j in range(grp):
                        lsl = slice(j * l, (j + 1) * l)
                        # per-partition IIR of this channel
                        nc.vector.tensor_tensor_scan(
                            g_t[:, lsl], a_tile[:], w_t[:, lsl], 0.0,
                            Alu.mult, Alu.add,
                        )
                        # carry[p] = g[p-1, L-1]; carry[0] = K0*w[0,0]
                        cp = psump.tile([p_used, 1], f32, tag="carry")
                        nc.tensor.matmul(
                            cp[:], shift_t[:], g_t[:, j * l + l - 1 : j * l + l],
                            start=True, stop=False,
                        )
                        nc.tensor.matmul(
                            cp[:], e0k0[:], w_t[0:1, j * l : j * l + 1],
                            start=False, stop=True,
                        )
                        carry_ps.append(cp)

                    # s1 = decay05*carry + waveform   (in-place onto wav)
                    for j in range(grp):
                        lsl = slice(j * l, (j + 1) * l)
                        nc.vector.scalar_tensor_tensor(
                            wav_t[:, lsl], decay_t[:], carry_ps[j][:, 0:1],
                            wav_t[:, lsl], Alu.mult, Alu.add,
                        )
                    # mixed = 0.05*b*g + s1   (whole group, in-place onto g)
                    nc.vector.scalar_tensor_tensor(
                        g_t[:], g_t[:], float(SC2), wav_t[:], Alu.mult, Alu.add
                    )
                    # per-group abs-max -> column g
                    nc.vector.tensor_reduce(
                        maxcols[:, g : g + 1], g_t[:], AxX, Alu.max,
                        apply_absolute_value=True,
                    )
                    md = dramp.tile([p_used, grp * l], f32, tag=f"mix{g}")
                    nc.gpsimd.dma_start(out=md[:], in_=g_t[:])
                    mixed_drams.append(md)

                # ---- global max + scale ----
                allmax = constp.tile([p_used, 1], f32, tag="allmax")
                nc.vector.tensor_reduce(
                    allmax[:], maxcols[:, 0:n_grp], AxX, Alu.max
                )
                gmax = constp.tile([p_used, 1], f32, tag="gmax")
                nc.gpsimd.partition_all_reduce(
                    gmax[:], allmax[:], channels=p_used,
                    reduce_op=bass_isa.ReduceOp.max,
                )
                sc_b = constp.tile([p_used, 1], f32, tag="scb")
                if n_cores > 1:
                    cc_in = dramp.tile([1, 1], f32, tag="ccin")
                    cc_out = dramp.tile([1, 1], f32, tag="ccout")
                    nc.sync.dma_start(out=cc_in[:], in_=gmax[0:1, 0:1])
                    nc.gpsimd.collective_compute(
                        "AllReduce",
                        Alu.max,
                        replica_groups=[list(range(n_cores))],
                        ins=[cc_in[:]],
                        outs=[cc_out[:]],
                    )
                    sc_small = constp.tile([1, 1], f32, tag="scsmall")
                    nc.sync.dma_start(out=sc_small[:], in_=cc_out[:])
                    nc.gpsimd.partition_broadcast(
                        sc_b[:], sc_small[0:1, 0:1], channels=p_used
                    )
                else:
                    nc.vector.tensor_copy(sc_b[:], gmax[:])
                # scale = 1 / max(gmax, 1.0)
                nc.vector.tensor_scalar_max(sc_b[:], sc_b[:], 1.0)
                inv_t = constp.tile([p_used, 1], f32, tag="inv")
                nc.vector.reciprocal(inv_t[:], sc_b[:])

            # ---- phase 2: rescale on ScalarE ----
            with tc.tile_pool(name="io2", bufs=3) as iop2:
                for g in range(n_grp):
                    cs = slice(g * grp, (g + 1) * grp)
                    m_t = iop2.tile([p_used, grp * l], f32, tag="m2")
                    nc.sync.dma_start(out=m_t[:], in_=mixed_drams[g][:])
                    o_t = iop2.tile([p_used, grp * l], f32, tag="o2")
                    nc.scalar.mul(o_t[:], m_t[:], inv_t[:, 0:1])
                    nc.scalar.dma_start(
                        out=out_pr[:, cs, :],
                        in_=o_t.rearrange("p (c l) -> p c l", c=grp),
                    )

    nc.compile()
    return nc


_CACHE = {}
LAST_RESULTS = None


def run(waveform, white_noise, c_per=C_PER, p_used=P_USED, l=L, n_cores=N_CORES,
        **spmd_kwargs):
    """Shard inputs over n_cores, run the SPMD bass kernel, gather output."""
    global LAST_RESULTS
    from concourse.bass_utils import run_bass_kernel_spmd

    key = (c_per, p_used, l, n_cores)
    if key not in _CACHE:
        _CACHE[key] = build_nc(c_per, p_used, l, n_cores)
    nc = _CACHE[key]

    decay, shift = _host_consts(p_used, l)
    waveform = np.ascontiguousarray(waveform, dtype=np.float32)
    white_noise = np.ascontiguousarray(white_noise, dtype=np.float32)

    in_maps = []
    for i in range(n_cores):
        sl = slice(i * c_per, (i + 1) * c_per)
        in_maps.append({
            "waveform": np.ascontiguousarray(waveform[sl]),
            "white_noise": np.ascontiguousarray(white_noise[sl]),
            "decay05": decay,
            "shiftmat": shift,
        })

    res = run_bass_kernel_spmd(nc, in_maps, core_ids=list(range(n_cores)),
                               **spmd_kwargs)
    LAST_RESULTS = res
    return np.concatenate([r["out"] for r in res.results], axis=0)


def kernel(waveform, white_noise):
    return run(waveform, white_noise)


# revision 8
# speedup vs baseline: 1.0966x; 1.0126x over previous
"""Trainium2 Bass kernel for nn_EnvironmentalAugmentations.

Computes, for waveform/white_noise of shape [256, 220500] fp32:
    pink  = first-order IIR of white_noise along time:
            f[0] = w[0];  f[t] = 0.99*f[t-1] + 0.01*w[t]
    mixed = waveform + 0.05 * pink
    out   = mixed / max(max|mixed|, 1.0)     (global max over all elements)

Strategy (8 NeuronCores, pure data-parallel over the 256 channels, 32/core):
  * Channels are processed in pairs: one SBUF tile [126 x 3500] holds
    channel A in partitions 0..62 and channel B in partitions 63..125,
    partition p covering 3500 consecutive samples (63 blocks per channel).
  * The IIR runs as ONE DVE `tensor_tensor_scan` per pair, in place on the
    input tile (state = a*state + w[t], zero init, per-partition).
  * Cross-partition carry: the true state entering block p is the last scan
    value of block p-1 (the a^3500 ~ 5e-16 remainder is far below fp32
    noise).  A PE matmul with a (channel-masked) superdiagonal shift matrix
    moves column 3499 down one partition; two accumulated rank-1 matmuls
    inject the t=0 initial-condition terms K0*w[0,0] (partition 0 resp. 63;
    the scan leaves column 0 unmodified, so reading w there is safe).
    A fused scalar_tensor_tensor computes s1 = decay05*carry + waveform
    (in place on the waveform tile); a second computes
    mixed = 0.05b*g + s1 (in place on the scan tile).
  * Global max: per-pair abs-max reduce -> column tile -> DVE reduce ->
    GPSIMD partition_all_reduce -> 8-core AllReduce(max) on a 4-byte DRAM
    scratch -> scale = 1/max(m,1) -> ScalarE (ACT) rescale pass (phase 2).
  * The last RES pairs stay resident in SBUF (no DRAM round-trip); earlier
    pairs spill to internal-DRAM scratch and are reloaded in phase 2.
  * DMA is split over the three descriptor engines (sync/scalar HWDGE +
    gpsimd SWDGE); every transfer is a contiguous-per-partition 1.76MB
    [126 x 14000B] block.
"""

import numpy as np

# fp32-rounded constants, mirrored from the reference
_A = float(np.float32(0.99))
_B = float(np.float32(0.01))
_NOISE = float(np.float32(0.05))

C_FULL, T_FULL = 256, 220500
N_CORES = 8
C_PER = C_FULL // N_CORES  # 32
P_USED = 126
L = T_FULL // P_USED       # 1750  (126*1750 == 220500 exactly)
RES = 6                    # channel pairs kept SBUF-resident through phase 2


def _host_consts(p_used, l):
    """Constants for the pair-stacked layout [p_used, 2*l]:
    decay05[p,k] = 0.05*b*a^(k+1); masked shift matrix; t=0 injectors."""
    a64, b64, n64 = _A, _B, _NOISE
    nb = p_used // 2
    lp = 2 * l
    k = np.arange(1, lp + 1, dtype=np.float64)
    decay_row = (n64 * b64) * np.power(a64, k)
    decay = np.ascontiguousarray(
        np.broadcast_to(decay_row.astype(np.float32), (p_used, lp))
    )
    shift = np.zeros((p_used, p_used), dtype=np.float32)
    for p in range(p_used - 1):
        if (p + 1) % nb != 0:
            shift[p, p + 1] = 1.0
    K0 = (1.0 - b64) / (b64 * a64)
    inj = np.zeros((p_used, p_used), dtype=np.float32)
    inj[0, 0] = K0
    inj[nb, nb] = K0
    return decay, shift, inj


def build_nc(c_per=C_PER, p_used=P_USED, l=L, n_cores=N_CORES, res=RES):
    """Build the Bacc module (per-core SPMD program)."""
    import concourse.mybir as mybir
    from concourse import bacc, bass_isa
    from concourse.tile import TileContext

    f32 = mybir.dt.float32
    Alu = mybir.AluOpType
    AxX = mybir.AxisListType.X
    t_loc = p_used * l
    assert p_used % 2 == 0
    assert c_per % 2 == 0
    nb = p_used // 2          # blocks per channel
    lp = 2 * l                # stacked row length
    n_grp = c_per // 2        # channel pairs
    res = min(res, n_grp)
    n_spill = n_grp - res

    SC2 = _NOISE * _B         # 0.05*b, the scan-output coefficient

    nc = bacc.Bacc(
        "TRN2", target_bir_lowering=False, debug=False, num_devices=n_cores
    )
    wave_h = nc.dram_tensor("waveform", [c_per, t_loc], f32, kind="ExternalInput")
    noise_h = nc.dram_tensor("white_noise", [c_per, t_loc], f32, kind="ExternalInput")
    decay_h = nc.dram_tensor("decay05", [p_used, lp], f32, kind="ExternalInput")
    shift_h = nc.dram_tensor("shiftmat", [p_used, p_used], f32, kind="ExternalInput")
    inj_h = nc.dram_tensor("injmat", [p_used, p_used], f32, kind="ExternalInput")
    out_h = nc.dram_tensor("out", [c_per, t_loc], f32, kind="ExternalOutput")

    # [(c nb), lp] views: row c*nb+p is block p of channel c (contiguous 14KB)
    wave_r = wave_h.rearrange("c (p l) -> (c p) l", p=nb)
    noise_r = noise_h.rearrange("c (p l) -> (c p) l", p=nb)
    out_r = out_h.rearrange("c (p l) -> (c p) l", p=nb)

    with TileContext(nc) as tc:
        with (
            tc.tile_pool(name="const", bufs=1) as constp,
            tc.tile_pool(name="dram", bufs=1, space="DRAM") as dramp,
        ):
            # constants (loaded on the scalar queue; sync starts data loads)
            a_small = constp.tile([p_used, 1], f32, tag="a_small")
            nc.gpsimd.memset(a_small[:], _A)
            a_bc = a_small.broadcast_to([p_used, lp])
            decay_t = constp.tile([p_used, lp], f32, tag="decay")
            nc.scalar.dma_start(out=decay_t[:], in_=decay_h[:, :])
            shift_t = constp.tile([p_used, p_used], f32, tag="shift")
            nc.scalar.dma_start(out=shift_t[:], in_=shift_h[:, :])
            inj_t = constp.tile([p_used, p_used], f32, tag="injmat")
            nc.scalar.dma_start(out=inj_t[:], in_=inj_h[:, :])
            maxcols = constp.tile([p_used, n_grp], f32, tag="maxcols")

            with (
                tc.tile_pool(name="io1", bufs=2) as iop,
                tc.tile_pool(name="resp", bufs=1) as resp,
                tc.tile_pool(name="psum", bufs=4, space="PSUM") as psump,
            ):
                spill_drams = []
                res_tiles = []
                for g in range(n_grp):
                    rows = slice(g * p_used, (g + 1) * p_used)
                    resident = g >= n_spill
                    if resident:
                        w_t = resp.tile([p_used, lp], f32, tag=f"res{g}")
                    else:
                        w_t = iop.tile([p_used, lp], f32, tag="w")
                    nc.sync.dma_start(out=w_t[:], in_=noise_r[rows, :])
                    wav_t = iop.tile([p_used, lp], f32, tag="wav")
                    nc.scalar.dma_start(out=wav_t[:], in_=wave_r[rows, :])

                    # in-place per-partition IIR (column 0 is left equal to w)
                    nc.vector.tensor_tensor_scan(
                        w_t[:], a_bc, w_t[:], 0.0, Alu.mult, Alu.add
                    )
                    # carry[p] = g[p-1, lp-1] (masked at channel boundary),
                    # plus K0*w[0,0] at partitions 0 and nb
                    cp = psump.tile([p_used, 1], f32, tag="carry")
                    nc.tensor.matmul(
                        cp[:], shift_t[:], w_t[:, lp - 1 : lp],
                        start=True, stop=False,
                    )
                    nc.tensor.matmul(
                        cp[:], inj_t[:], w_t[:, 0:1], start=False, stop=True,
                    )

                    # s1 = decay05*carry + waveform   (in-place onto wav)
                    nc.vector.scalar_tensor_tensor(
                        wav_t[:], decay_t[:], cp[:, 0:1], wav_t[:],
                        Alu.mult, Alu.add,
                    )
                    # mixed = 0.05b*g + s1   (in-place onto scan tile)
                    nc.vector.scalar_tensor_tensor(
                        w_t[:], w_t[:], float(SC2), wav_t[:], Alu.mult, Alu.add
                    )
                    # per-pair abs-max
                    nc.vector.tensor_reduce(
                        maxcols[:, g : g + 1], w_t[:], AxX, Alu.max,
                        apply_absolute_value=True,
                    )
                    if resident:
                        res_tiles.append(w_t)
                    else:
                        md = dramp.tile([p_used, lp], f32, tag=f"mix{g}")
                        nc.gpsimd.dma_start(out=md[:], in_=w_t[:])
                        spill_drams.append(md)

                # ---- global max + scale ----
                allmax = constp.tile([p_used, 1], f32, tag="allmax")
                nc.vector.tensor_reduce(
                    allmax[:], maxcols[:, 0:n_grp], AxX, Alu.max
                )
                gmax = constp.tile([p_used, 1], f32, tag="gmax")
                nc.gpsimd.partition_all_reduce(
                    gmax[:], allmax[:], channels=p_used,
                    reduce_op=bass_isa.ReduceOp.max,
                )
                sc_b = constp.tile([p_used, 1], f32, tag="scb")
                if n_cores > 1:
                    cc_in = dramp.tile([1, 1], f32, tag="ccin")
                    cc_out = dramp.tile([1, 1], f32, tag="ccout")
                    nc.sync.dma_start(out=cc_in[:], in_=gmax[0:1, 0:1])
                    nc.gpsimd.collective_compute(
                        "AllReduce",
                        Alu.max,
                        replica_groups=[list(range(n_cores))],
                        ins=[cc_in[:]],
                        outs=[cc_out[:]],
                    )
                    sc_small = constp.tile([1, 1], f32, tag="scsmall")
                    nc.sync.dma_start(out=sc_small[:], in_=cc_out[:])
                    nc.gpsimd.partition_broadcast(
                        sc_b[:], sc_small[0:1, 0:1], channels=p_used
                    )
                else:
                    nc.vector.tensor_copy(sc_b[:], gmax[:])
                # scale = 1 / max(gmax, 1.0)
                nc.vector.tensor_scalar_max(sc_b[:], sc_b[:], 1.0)
                inv_t = constp.tile([p_used, 1], f32, tag="inv")
                nc.vector.reciprocal(inv_t[:], sc_b[:])

                # ---- phase 2: rescale on ScalarE (ACT) ----
                with tc.tile_pool(name="io2", bufs=2) as iop2:
                    for g in range(n_grp):
                        rows = slice(g * p_used, (g + 1) * p_used)
                        if g >= n_spill:
                            t = res_tiles[g - n_spill]
                            nc.scalar.mul(t[:], t[:], inv_t[:, 0:1])
                            dma = nc.gpsimd if g % 2 == 0 else nc.scalar
                            dma.dma_start(out=out_r[rows, :], in_=t[:])
                        else:
                            m_t = iop2.tile([p_used, lp], f32, tag="m2")
                            nc.sync.dma_start(
                                out=m_t[:], in_=spill_drams[g][:]
                            )
                            nc.scalar.mul(m_t[:], m_t[:], inv_t[:, 0:1])
                            dma = nc.gpsimd if g % 2 == 0 else nc.scalar
                            dma.dma_start(out=out_r[rows, :], in_=m_t[:])

    nc.compile()
    return nc


_CACHE = {}
LAST_RESULTS = None


def run(waveform, white_noise, c_per=C_PER, p_used=P_USED, l=L, n_cores=N_CORES,
        **spmd_kwargs):
    """Shard inputs over n_cores, run the SPMD bass kernel, gather output."""
    global LAST_RESULTS
    from concourse.bass_utils import run_bass_kernel_spmd

    key = (c_per, p_used, l, n_cores)
    if key not in _CACHE:
        _CACHE[key] = build_nc(c_per, p_used, l, n_cores)
    nc = _CACHE[key]

    decay, shift, inj = _host_consts(p_used, l)
    waveform = np.ascontiguousarray(waveform, dtype=np.float32)
    white_noise = np.ascontiguousarray(white_noise, dtype=np.float32)

    in_maps = []
    for i in range(n_cores):
        sl = slice(i * c_per, (i + 1) * c_per)
        in_maps.append({
            "waveform": np.ascontiguousarray(waveform[sl]),
            "white_noise": np.ascontiguousarray(white_noise[sl]),
            "decay05": decay,
            "shiftmat": shift,
            "injmat": inj,
        })

    res = run_bass_kernel_spmd(nc, in_maps, core_ids=list(range(n_cores)),
                               **spmd_kwargs)
    LAST_RESULTS = res
    return np.concatenate([r["out"] for r in res.results], axis=0)


def kernel(waveform, white_noise):
    return run(waveform, white_noise)


# revision 9
# speedup vs baseline: 1.1156x; 1.0173x over previous
"""Trainium2 Bass kernel for nn_EnvironmentalAugmentations.

Computes, for waveform/white_noise of shape [256, 220500] fp32:
    pink  = first-order IIR of white_noise along time:
            f[0] = w[0];  f[t] = 0.99*f[t-1] + 0.01*w[t]
    mixed = waveform + 0.05 * pink
    out   = mixed / max(max|mixed|, 1.0)     (global max over all elements)

Strategy (8 NeuronCores, pure data-parallel over the 256 channels, 32/core):
  * Channels are processed in pairs: one SBUF tile [126 x 3500] holds
    channel A in partitions 0..62 and channel B in partitions 63..125,
    partition p covering 3500 consecutive samples (63 blocks per channel).
  * The IIR runs as ONE DVE `tensor_tensor_scan` per pair, in place on the
    input tile (state = a*state + w[t], zero init, per-partition).
  * Cross-partition carry: the true state entering block p is the last scan
    value of block p-1 (the a^3500 ~ 5e-16 remainder is far below fp32
    noise).  A PE matmul with a (channel-masked) superdiagonal shift matrix
    moves column 3499 down one partition; two accumulated rank-1 matmuls
    inject the t=0 initial-condition terms K0*w[0,0] (partition 0 resp. 63;
    the scan leaves column 0 unmodified, so reading w there is safe).
    A fused scalar_tensor_tensor computes s1 = decay05*carry + waveform
    (in place on the waveform tile); a second computes
    mixed = 0.05b*g + s1 (in place on the scan tile).
  * Global max: per-pair abs-max reduce -> column tile -> DVE reduce ->
    GPSIMD partition_all_reduce -> 8-core AllReduce(max) on a 4-byte DRAM
    scratch -> scale = 1/max(m,1) -> ScalarE (ACT) rescale pass (phase 2).
  * The last RES pairs stay resident in SBUF (no DRAM round-trip); earlier
    pairs spill to internal-DRAM scratch and are reloaded in phase 2.
  * DMA is split over the three descriptor engines (sync/scalar HWDGE +
    gpsimd SWDGE); every transfer is a contiguous-per-partition 1.76MB
    [126 x 14000B] block.
"""

import numpy as np

# fp32-rounded constants, mirrored from the reference
_A = float(np.float32(0.99))
_B = float(np.float32(0.01))
_NOISE = float(np.float32(0.05))

C_FULL, T_FULL = 256, 220500
N_CORES = 8
C_PER = C_FULL // N_CORES  # 32
P_USED = 126
L = T_FULL // P_USED       # 1750  (126*1750 == 220500 exactly)
RES = 5                    # channel pairs kept SBUF-resident through phase 2


def _host_consts(p_used, l):
    """Constants for the pair-stacked layout [p_used, 2*l]:
    decay05[p,k] = 0.05*b*a^(k+1); masked shift matrix; t=0 injectors."""
    a64, b64, n64 = _A, _B, _NOISE
    nb = p_used // 2
    lp = 2 * l
    k = np.arange(1, lp + 1, dtype=np.float64)
    decay_row = (n64 * b64) * np.power(a64, k)
    decay = np.ascontiguousarray(
        np.broadcast_to(decay_row.astype(np.float32), (p_used, lp))
    )
    shift = np.zeros((p_used, p_used), dtype=np.float32)
    for p in range(p_used - 1):
        if (p + 1) % nb != 0:
            shift[p, p + 1] = 1.0
    K0 = (1.0 - b64) / (b64 * a64)
    inj = np.zeros((p_used, p_used), dtype=np.float32)
    inj[0, 0] = K0
    inj[nb, nb] = K0
    return decay, shift, inj


def build_nc(c_per=C_PER, p_used=P_USED, l=L, n_cores=N_CORES, res=RES):
    """Build the Bacc module (per-core SPMD program)."""
    import concourse.mybir as mybir
    from concourse import bacc, bass_isa
    from concourse.tile import TileContext

    f32 = mybir.dt.float32
    Alu = mybir.AluOpType
    AxX = mybir.AxisListType.X
    t_loc = p_used * l
    assert p_used % 2 == 0
    assert c_per % 2 == 0
    nb = p_used // 2          # blocks per channel
    lp = 2 * l                # stacked row length
    n_grp = c_per // 2        # channel pairs
    res = min(res, n_grp)
    n_spill = n_grp - res

    SC2 = _NOISE * _B         # 0.05*b, the scan-output coefficient

    nc = bacc.Bacc(
        "TRN2", target_bir_lowering=False, debug=False, num_devices=n_cores
    )
    wave_h = nc.dram_tensor("waveform", [c_per, t_loc], f32, kind="ExternalInput")
    noise_h = nc.dram_tensor("white_noise", [c_per, t_loc], f32, kind="ExternalInput")
    decay_h = nc.dram_tensor("decay05", [p_used, lp], f32, kind="ExternalInput")
    shift_h = nc.dram_tensor("shiftmat", [p_used, p_used], f32, kind="ExternalInput")
    inj_h = nc.dram_tensor("injmat", [p_used, p_used], f32, kind="ExternalInput")
    out_h = nc.dram_tensor("out", [c_per, t_loc], f32, kind="ExternalOutput")

    # [(c nb), lp] views: row c*nb+p is block p of channel c (contiguous 14KB)
    wave_r = wave_h.rearrange("c (p l) -> (c p) l", p=nb)
    noise_r = noise_h.rearrange("c (p l) -> (c p) l", p=nb)
    out_r = out_h.rearrange("c (p l) -> (c p) l", p=nb)

    with TileContext(nc) as tc:
        with (
            tc.tile_pool(name="const", bufs=1) as constp,
            tc.tile_pool(name="dram", bufs=1, space="DRAM") as dramp,
        ):
            # constants (loaded on the scalar queue; sync starts data loads)
            a_small = constp.tile([p_used, 1], f32, tag="a_small")
            nc.gpsimd.memset(a_small[:], _A)
            a_bc = a_small.broadcast_to([p_used, lp])
            decay_t = constp.tile([p_used, lp], f32, tag="decay")
            nc.scalar.dma_start(out=decay_t[:], in_=decay_h[:, :])
            shift_t = constp.tile([p_used, p_used], f32, tag="shift")
            nc.scalar.dma_start(out=shift_t[:], in_=shift_h[:, :])
            inj_t = constp.tile([p_used, p_used], f32, tag="injmat")
            nc.scalar.dma_start(out=inj_t[:], in_=inj_h[:, :])
            maxcols = constp.tile([p_used, n_grp], f32, tag="maxcols")

            with (
                tc.tile_pool(name="io1", bufs=3) as iop,
                tc.tile_pool(name="resp", bufs=1) as resp,
                tc.tile_pool(name="psum", bufs=4, space="PSUM") as psump,
            ):
                spill_drams = []
                res_tiles = []
                for g in range(n_grp):
                    rows = slice(g * p_used, (g + 1) * p_used)
                    resident = g >= n_spill
                    if resident:
                        w_t = resp.tile([p_used, lp], f32, tag=f"res{g}")
                    else:
                        w_t = iop.tile([p_used, lp], f32, tag="w")
                    nc.sync.dma_start(out=w_t[:], in_=noise_r[rows, :])
                    wav_t = iop.tile([p_used, lp], f32, tag="wav")
                    nc.scalar.dma_start(out=wav_t[:], in_=wave_r[rows, :])

                    # in-place per-partition IIR (column 0 is left equal to w)
                    nc.vector.tensor_tensor_scan(
                        w_t[:], a_bc, w_t[:], 0.0, Alu.mult, Alu.add
                    )
                    # carry[p] = g[p-1, lp-1] (masked at channel boundary),
                    # plus K0*w[0,0] at partitions 0 and nb
                    cp = psump.tile([p_used, 1], f32, tag="carry")
                    nc.tensor.matmul(
                        cp[:], shift_t[:], w_t[:, lp - 1 : lp],
                        start=True, stop=False,
                    )
                    nc.tensor.matmul(
                        cp[:], inj_t[:], w_t[:, 0:1], start=False, stop=True,
                    )

                    # s1 = decay05*carry + waveform   (in-place onto wav)
                    nc.vector.scalar_tensor_tensor(
                        wav_t[:], decay_t[:], cp[:, 0:1], wav_t[:],
                        Alu.mult, Alu.add,
                    )
                    # mixed = 0.05b*g + s1   (in-place onto scan tile)
                    nc.vector.scalar_tensor_tensor(
                        w_t[:], w_t[:], float(SC2), wav_t[:], Alu.mult, Alu.add
                    )
                    # per-pair abs-max
                    nc.vector.tensor_reduce(
                        maxcols[:, g : g + 1], w_t[:], AxX, Alu.max,
                        apply_absolute_value=True,
                    )
                    if resident:
                        res_tiles.append(w_t)
                    else:
                        md = dramp.tile([p_used, lp], f32, tag=f"mix{g}")
                        sdma = nc.sync if g % 2 == 0 else nc.scalar
                        sdma.dma_start(out=md[:], in_=w_t[:])
                        spill_drams.append(md)

                # ---- global max + scale ----
                allmax = constp.tile([p_used, 1], f32, tag="allmax")
                nc.vector.tensor_reduce(
                    allmax[:], maxcols[:, 0:n_grp], AxX, Alu.max
                )
                gmax = constp.tile([p_used, 1], f32, tag="gmax")
                nc.gpsimd.partition_all_reduce(
                    gmax[:], allmax[:], channels=p_used,
                    reduce_op=bass_isa.ReduceOp.max,
                )
                sc_b = constp.tile([p_used, 1], f32, tag="scb")
                if n_cores > 1:
                    cc_in = dramp.tile([1, 1], f32, tag="ccin")
                    cc_out = dramp.tile([1, 1], f32, tag="ccout")
                    nc.sync.dma_start(out=cc_in[:], in_=gmax[0:1, 0:1])
                    nc.gpsimd.collective_compute(
                        "AllReduce",
                        Alu.max,
                        replica_groups=[list(range(n_cores))],
                        ins=[cc_in[:]],
                        outs=[cc_out[:]],
                    )
                    sc_small = constp.tile([1, 1], f32, tag="scsmall")
                    nc.sync.dma_start(out=sc_small[:], in_=cc_out[:])
                    nc.gpsimd.partition_broadcast(
                        sc_b[:], sc_small[0:1, 0:1], channels=p_used
                    )
                else:
                    nc.vector.tensor_copy(sc_b[:], gmax[:])
                # scale = 1 / max(gmax, 1.0)
                nc.vector.tensor_scalar_max(sc_b[:], sc_b[:], 1.0)
                inv_t = constp.tile([p_used, 1], f32, tag="inv")
                nc.vector.reciprocal(inv_t[:], sc_b[:])

                # ---- phase 2: rescale (DVE for residents, ACT for spills) ----
                with tc.tile_pool(name="io2", bufs=2) as iop2:
                    for i, g in enumerate(range(n_spill, n_grp)):
                        rows = slice(g * p_used, (g + 1) * p_used)
                        t = res_tiles[g - n_spill]
                        nc.vector.tensor_scalar_mul(t[:], t[:], inv_t[:, 0:1])
                        dma = nc.gpsimd if i % 2 == 0 else nc.scalar
                        dma.dma_start(out=out_r[rows, :], in_=t[:])
                    for g in range(n_spill):
                        rows = slice(g * p_used, (g + 1) * p_used)
                        m_t = iop2.tile([p_used, lp], f32, tag="m2")
                        nc.sync.dma_start(out=m_t[:], in_=spill_drams[g][:])
                        nc.scalar.mul(m_t[:], m_t[:], inv_t[:, 0:1])
                        dma = nc.gpsimd if g % 2 == 0 else nc.scalar
                        dma.dma_start(out=out_r[rows, :], in_=m_t[:])

    nc.compile()
    return nc


_CACHE = {}
LAST_RESULTS = None


def run(waveform, white_noise, c_per=C_PER, p_used=P_USED, l=L, n_cores=N_CORES,
        **spmd_kwargs):
    """Shard inputs over n_cores, run the SPMD bass kernel, gather output."""
    global LAST_RESULTS
    from concourse.bass_utils import run_bass_kernel_spmd

    key = (c_per, p_used, l, n_cores)
    if key not in _CACHE:
        _CACHE[key] = build_nc(c_per, p_used, l, n_cores)
    nc = _CACHE[key]

    decay, shift, inj = _host_consts(p_used, l)
    waveform = np.ascontiguousarray(waveform, dtype=np.float32)
    white_noise = np.ascontiguousarray(white_noise, dtype=np.float32)

    in_maps = []
    for i in range(n_cores):
        sl = slice(i * c_per, (i + 1) * c_per)
        in_maps.append({
            "waveform": np.ascontiguousarray(waveform[sl]),
            "white_noise": np.ascontiguousarray(white_noise[sl]),
            "decay05": decay,
            "shiftmat": shift,
            "injmat": inj,
        })

    res = run_bass_kernel_spmd(nc, in_maps, core_ids=list(range(n_cores)),
                               **spmd_kwargs)
    LAST_RESULTS = res
    return np.concatenate([r["out"] for r in res.results], axis=0)


def kernel(waveform, white_noise):
    return run(waveform, white_noise)


# revision 12
# speedup vs baseline: 1.1525x; 1.0331x over previous
"""Trainium2 Bass kernel for nn_EnvironmentalAugmentations.

Computes, for waveform/white_noise of shape [256, 220500] fp32:
    pink  = first-order IIR of white_noise along time:
            f[0] = w[0];  f[t] = 0.99*f[t-1] + 0.01*w[t]
    mixed = waveform + 0.05 * pink
    out   = mixed / max(max|mixed|, 1.0)     (global max over all elements)

Strategy (8 NeuronCores, pure data-parallel over the 256 channels, 32/core):
  * Channels are processed in pairs: one SBUF tile [126 x 3500] holds
    channel A in partitions 0..62 and channel B in partitions 63..125,
    partition p covering 3500 consecutive samples (63 blocks per channel).
  * The IIR runs as ONE DVE `tensor_tensor_scan` per pair, in place on the
    input tile (state = a*state + w[t], zero init, per-partition).
  * Cross-partition carry: the true state entering block p is the last scan
    value of block p-1 (the a^3500 ~ 5e-16 remainder is far below fp32
    noise).  PE matmuls build the carry column in PSUM: a channel-masked
    superdiagonal shift matrix moves scan column 3499 down one partition,
    and a diagonal injector adds the t=0 initial-condition terms
    K0*w[0,0] at partitions 0 and 63 (the scan leaves column 0 equal to w).
  * The mix runs mostly on the otherwise-idle PE/ACT engines:
    carry column -> SBUF (ACT) -> PE transpose -> carry row (ACT), then PE
    accumulates  s1 = I @ waveform + carry_row (x) decay05_row  into PSUM;
    one DVE scalar_tensor_tensor computes mixed = 0.05b*g + s1 per chunk.
  * Global max: per-pair abs-max reduce -> column tile -> DVE reduce ->
    GPSIMD partition_all_reduce -> 8-core AllReduce(max) on a 4-byte DRAM
    scratch -> scale = 1/max(m,1); phase 2 rescales (DVE for resident
    pairs, ACT for spilled ones).
  * The last RES pairs stay resident in SBUF (no DRAM round-trip); earlier
    pairs spill to internal-DRAM scratch and are reloaded in phase 2.
  * DMA is split over the three descriptor engines (sync/scalar HWDGE +
    gpsimd SWDGE); every transfer is a contiguous-per-partition 1.76MB
    [126 x 14000B] block.
"""

import numpy as np

# fp32-rounded constants, mirrored from the reference
_A = float(np.float32(0.99))
_B = float(np.float32(0.01))
_NOISE = float(np.float32(0.05))

C_FULL, T_FULL = 256, 220500
N_CORES = 8
C_PER = C_FULL // N_CORES  # 32
P_USED = 126
L = T_FULL // P_USED       # 1750  (126*1750 == 220500 exactly)
RES = 6                    # channel pairs kept SBUF-resident through phase 2
PE_MIX = False             # PE-PSUM mix path hits a HW fault; use DVE STT mix
QCH = 875                  # PSUM chunk width for the PE-mix (2 banks)


def _host_consts(p_used, l):
    """Constants for the pair-stacked layout [p_used, 2*l]."""
    a64, b64, n64 = _A, _B, _NOISE
    nb = p_used // 2
    lp = 2 * l
    k = np.arange(1, lp + 1, dtype=np.float64)
    decay_row = ((n64 * b64) * np.power(a64, k)).astype(np.float32)
    decay_row = np.ascontiguousarray(decay_row[None, :])      # [1, lp]
    shift = np.zeros((p_used, p_used), dtype=np.float32)
    for p in range(p_used - 1):
        if (p + 1) % nb != 0:
            shift[p, p + 1] = 1.0
    K0 = (1.0 - b64) / (b64 * a64)
    inj = np.zeros((p_used, p_used), dtype=np.float32)
    inj[0, 0] = K0
    inj[nb, nb] = K0
    ident = np.eye(p_used, dtype=np.float32)
    return decay_row, shift, inj, ident


def build_nc(c_per=C_PER, p_used=P_USED, l=L, n_cores=N_CORES, res=RES,
             pe_mix=None):
    """Build the Bacc module (per-core SPMD program)."""
    import concourse.mybir as mybir
    from concourse import bacc, bass_isa
    from concourse.tile import TileContext

    f32 = mybir.dt.float32
    Alu = mybir.AluOpType
    AxX = mybir.AxisListType.X
    t_loc = p_used * l
    assert p_used % 2 == 0
    assert c_per % 2 == 0
    nb = p_used // 2          # blocks per channel
    lp = 2 * l                # stacked row length
    n_grp = c_per // 2        # channel pairs
    res = min(res, n_grp)
    if pe_mix is None:
        pe_mix = PE_MIX
    n_spill = n_grp - res

    SC2 = _NOISE * _B         # 0.05*b, the scan-output coefficient
    # PSUM mix chunks: quarters of lp, each split into <=512 matmul pieces
    qch = min(QCH, lp)
    assert lp % qch == 0
    nq = lp // qch

    nc = bacc.Bacc(
        "TRN2", target_bir_lowering=False, debug=False, num_devices=n_cores
    )
    wave_h = nc.dram_tensor("waveform", [c_per, t_loc], f32, kind="ExternalInput")
    noise_h = nc.dram_tensor("white_noise", [c_per, t_loc], f32, kind="ExternalInput")
    decay_h = nc.dram_tensor("decayrow", [1, lp], f32, kind="ExternalInput")
    shift_h = nc.dram_tensor("shiftmat", [p_used, p_used], f32, kind="ExternalInput")
    inj_h = nc.dram_tensor("injmat", [p_used, p_used], f32, kind="ExternalInput")
    ident_h = nc.dram_tensor("identmat", [p_used, p_used], f32, kind="ExternalInput")
    out_h = nc.dram_tensor("out", [c_per, t_loc], f32, kind="ExternalOutput")

    # [(c nb), lp] views: row c*nb+p is block p of channel c (contiguous 14KB)
    wave_r = wave_h.rearrange("c (p l) -> (c p) l", p=nb)
    noise_r = noise_h.rearrange("c (p l) -> (c p) l", p=nb)
    out_r = out_h.rearrange("c (p l) -> (c p) l", p=nb)

    with TileContext(nc) as tc:
        with (
            tc.tile_pool(name="const", bufs=1) as constp,
            tc.tile_pool(name="dram", bufs=1, space="DRAM") as dramp,
        ):
            # constants (scalar queue; sync starts data loads)
            a_small = constp.tile([p_used, 1], f32, tag="a_small")
            nc.gpsimd.memset(a_small[:], _A)
            a_bc = a_small.broadcast_to([p_used, lp])
            decay_t = constp.tile([1, lp], f32, tag="decayrow")
            nc.scalar.dma_start(out=decay_t[:], in_=decay_h[:, :])
            shift_t = constp.tile([p_used, p_used], f32, tag="shift")
            nc.scalar.dma_start(out=shift_t[:], in_=shift_h[:, :])
            inj_t = constp.tile([p_used, p_used], f32, tag="injmat")
            nc.scalar.dma_start(out=inj_t[:], in_=inj_h[:, :])
            ident_t = constp.tile([p_used, p_used], f32, tag="identmat")
            nc.scalar.dma_start(out=ident_t[:], in_=ident_h[:, :])
            maxcols = constp.tile([p_used, n_grp], f32, tag="maxcols")
            if not pe_mix:
                decay_full = constp.tile([p_used, lp], f32, tag="decayfull")
                nc.gpsimd.partition_broadcast(
                    decay_full[:], decay_t[0:1, :], channels=p_used
                )

            with (
                tc.tile_pool(name="io1", bufs=3) as iop,
                tc.tile_pool(name="wavp", bufs=2) as wavp,
                tc.tile_pool(name="resp", bufs=1) as resp,
                tc.tile_pool(name="cps", bufs=2, space="PSUM") as cpsp,
                tc.tile_pool(name="mixps", bufs=2, space="PSUM") as mixps,
                tc.tile_pool(name="rowp", bufs=2) as rowp,
            ):
                spill_drams = []
                res_tiles = []
                for g in range(n_grp):
                    rows = slice(g * p_used, (g + 1) * p_used)
                    resident = g >= n_spill
                    if resident:
                        w_t = resp.tile([p_used, lp], f32, tag=f"res{g}")
                    else:
                        w_t = iop.tile([p_used, lp], f32, tag="w")
                    nc.sync.dma_start(out=w_t[:], in_=noise_r[rows, :])
                    wav_t = wavp.tile([p_used, lp], f32, tag="wav")
                    nc.scalar.dma_start(out=wav_t[:], in_=wave_r[rows, :])

                    # in-place per-partition IIR (column 0 stays equal to w)
                    nc.vector.tensor_tensor_scan(
                        w_t[:], a_bc, w_t[:], 0.0, Alu.mult, Alu.add
                    )
                    # carry column in PSUM
                    cp = cpsp.tile([p_used, 1], f32, tag="carry")
                    nc.tensor.matmul(
                        cp[:], shift_t[:], w_t[:, lp - 1 : lp],
                        start=True, stop=False,
                    )
                    nc.tensor.matmul(
                        cp[:], inj_t[:], w_t[:, 0:1], start=False, stop=True,
                    )
                    if pe_mix:
                        # carry column -> SBUF -> PE transpose -> carry row
                        ccol = rowp.tile([p_used, 1], f32, tag="ccol")
                        nc.scalar.copy(ccol[:], cp[:, 0:1])
                        crow_ps = cpsp.tile([1, p_used], f32, tag="crow_ps")
                        nc.tensor.transpose(crow_ps[:], ccol[:], ident_t[:])
                        crow = rowp.tile([1, p_used], f32, tag="crow")
                        nc.scalar.copy(crow[:], crow_ps[:, :])

                        # s1 = I @ waveform + carry_row (x) decay05_row (PSUM)
                        # then mixed = 0.05b*g + s1 (DVE STT, in-place onto w)
                        for q in range(nq):
                            q0 = q * qch
                            s1q = mixps.tile([p_used, qch], f32, tag="s1q")
                            s = 0
                            while s < qch:
                                e = min(s + 512, qch)
                                nc.tensor.matmul(
                                    s1q[:, s:e], ident_t[:],
                                    wav_t[:, q0 + s : q0 + e],
                                    start=True, stop=False,
                                )
                                nc.tensor.matmul(
                                    s1q[:, s:e], crow[:],
                                    decay_t[:, q0 + s : q0 + e],
                                    start=False, stop=True,
                                )
                                s = e
                            nc.vector.scalar_tensor_tensor(
                                w_t[:, q0 : q0 + qch], w_t[:, q0 : q0 + qch],
                                float(SC2), s1q[:], Alu.mult, Alu.add,
                            )
                    else:
                        # s1 = decay05*carry + waveform (in-place onto wav)
                        nc.vector.scalar_tensor_tensor(
                            wav_t[:], decay_full[:], cp[:, 0:1], wav_t[:],
                            Alu.mult, Alu.add,
                        )
                        # mixed = 0.05b*g + s1 (in-place onto scan tile)
                        nc.vector.scalar_tensor_tensor(
                            w_t[:], w_t[:], float(SC2), wav_t[:],
                            Alu.mult, Alu.add,
                        )
                    # per-pair abs-max
                    nc.vector.tensor_reduce(
                        maxcols[:, g : g + 1], w_t[:], AxX, Alu.max,
                        apply_absolute_value=True,
                    )
                    if resident:
                        res_tiles.append(w_t)
                    else:
                        md = dramp.tile([p_used, lp], f32, tag=f"mix{g}")
                        sdma = nc.sync if g % 2 == 0 else nc.scalar
                        sdma.dma_start(out=md[:], in_=w_t[:])
                        spill_drams.append(md)

                # ---- global max + scale ----
                allmax = constp.tile([p_used, 1], f32, tag="allmax")
                nc.vector.tensor_reduce(
                    allmax[:], maxcols[:, 0:n_grp], AxX, Alu.max
                )
                gmax = constp.tile([p_used, 1], f32, tag="gmax")
                nc.gpsimd.partition_all_reduce(
                    gmax[:], allmax[:], channels=p_used,
                    reduce_op=bass_isa.ReduceOp.max,
                )
                sc_b = constp.tile([p_used, 1], f32, tag="scb")
                if n_cores > 1:
                    cc_in = dramp.tile([1, 1], f32, tag="ccin")
                    cc_out = dramp.tile([1, 1], f32, tag="ccout")
                    nc.sync.dma_start(out=cc_in[:], in_=gmax[0:1, 0:1])
                    nc.gpsimd.collective_compute(
                        "AllReduce",
                        Alu.max,
                        replica_groups=[list(range(n_cores))],
                        ins=[cc_in[:]],
                        outs=[cc_out[:]],
                    )
                    sc_small = constp.tile([1, 1], f32, tag="scsmall")
                    nc.sync.dma_start(out=sc_small[:], in_=cc_out[:])
                    nc.gpsimd.partition_broadcast(
                        sc_b[:], sc_small[0:1, 0:1], channels=p_used
                    )
                else:
                    nc.vector.tensor_copy(sc_b[:], gmax[:])
                # scale = 1 / max(gmax, 1.0)
                nc.vector.tensor_scalar_max(sc_b[:], sc_b[:], 1.0)
                inv_t = constp.tile([p_used, 1], f32, tag="inv")
                nc.vector.reciprocal(inv_t[:], sc_b[:])

                # ---- phase 2: rescale (DVE for residents, ACT for spills) ----
                with tc.tile_pool(name="io2", bufs=2) as iop2:
                    for i, g in enumerate(range(n_spill, n_grp)):
                        rows = slice(g * p_used, (g + 1) * p_used)
                        t = res_tiles[g - n_spill]
                        nc.vector.tensor_scalar_mul(t[:], t[:], inv_t[:, 0:1])
                        dma = nc.gpsimd if i % 2 == 0 else nc.scalar
                        dma.dma_start(out=out_r[rows, :], in_=t[:])
                    for g in range(n_spill):
                        rows = slice(g * p_used, (g + 1) * p_used)
                        m_t = iop2.tile([p_used, lp], f32, tag="m2")
                        nc.sync.dma_start(out=m_t[:], in_=spill_drams[g][:])
                        nc.scalar.mul(m_t[:], m_t[:], inv_t[:, 0:1])
                        dma = nc.gpsimd if g % 2 == 0 else nc.scalar
                        dma.dma_start(out=out_r[rows, :], in_=m_t[:])

    nc.compile()
    return nc


_CACHE = {}
LAST_RESULTS = None


def run(waveform, white_noise, c_per=C_PER, p_used=P_USED, l=L, n_cores=N_CORES,
        **spmd_kwargs):
    """Shard inputs over n_cores, run the SPMD bass kernel, gather output."""
    global LAST_RESULTS
    from concourse.bass_utils import run_bass_kernel_spmd

    key = (c_per, p_used, l, n_cores)
    if key not in _CACHE:
        _CACHE[key] = build_nc(c_per, p_used, l, n_cores)
    nc = _CACHE[key]

    decay_row, shift, inj, ident = _host_consts(p_used, l)
    waveform = np.ascontiguousarray(waveform, dtype=np.float32)
    white_noise = np.ascontiguousarray(white_noise, dtype=np.float32)

    in_maps = []
    for i in range(n_cores):
        sl = slice(i * c_per, (i + 1) * c_per)
        in_maps.append({
            "waveform": np.ascontiguousarray(waveform[sl]),
            "white_noise": np.ascontiguousarray(white_noise[sl]),
            "decayrow": decay_row,
            "shiftmat": shift,
            "injmat": inj,
            "identmat": ident,
        })

    res = run_bass_kernel_spmd(nc, in_maps, core_ids=list(range(n_cores)),
                               **spmd_kwargs)
    LAST_RESULTS = res
    return np.concatenate([r["out"] for r in res.results], axis=0)


def kernel(waveform, white_noise):
    return run(waveform, white_noise)
